# revision 56
# baseline (speedup 1.0000x reference)
"""Self-contained Trainium2 Bass kernel for the multi-head attention module.

Sharding: flat 8-way head tensor-parallelism. Core c owns heads {2c, 2c+1}
for both batches; after attention one 8-core AllToAll reshards from
head-space to sequence-space and each core runs the output projection for
its 512 token rows. Host concatenates the per-core row chunks.

All matmuls run in float32r (TF32-like, 1 cyc/row for moving dim >= 256).
"""

import sys

sys.path.insert(0, "/opt/trn_rl_repo")

import numpy as np

from concourse import bacc, bass_utils, mybir, tile
from concourse.masks import make_identity

B, S, D, H, DK, DV, DO = 2, 2048, 1024, 16, 64, 64, 1024
T = B * S          # 4096 flattened tokens
NCORES = 8
HPC = H // NCORES  # 2 heads per core
ROWS = T // NCORES # 512 output rows per core
TCH = 512          # token chunk for projections / q chunks
F32 = mybir.dt.float32
F32R = mybir.dt.float32r
EXP = mybir.ActivationFunctionType.Exp

_cache = {}


def _build(collective=True, phases=4):
    nc = bacc.Bacc("TRN2", target_bir_lowering=False, debug=False,
                   num_devices=NCORES if collective else 1)
    x_d = nc.dram_tensor("x", [T, D], F32R, kind="ExternalInput").ap()
    wq_d = nc.dram_tensor("wq", [D, HPC * DK], F32R, kind="ExternalInput").ap()
    wk_d = nc.dram_tensor("wk", [D, HPC * DK], F32R, kind="ExternalInput").ap()
    wv_d = nc.dram_tensor("wv", [D, HPC * DV], F32R, kind="ExternalInput").ap()
    wo_d = nc.dram_tensor("wo", [H * DV, DO], F32R, kind="ExternalInput").ap()
    out_d = nc.dram_tensor("out", [ROWS, DO], F32, kind="ExternalOutput").ap()
    bnc_in = [nc.dram_tensor(f"bnc_in{h}", [NCORES, 64, ROWS], F32R).ap()
              for h in range(HPC)]
    bnc_out = [nc.dram_tensor(f"bnc_out{h}", [NCORES, 64, ROWS], F32R).ap()
               for h in range(HPC)]

    with tile.TileContext(nc) as tc:
        with (
            tc.tile_pool(name="sb", bufs=1) as sb,
            tc.tile_pool(name="ps", bufs=1, space="PSUM") as ps,
            nc.allow_low_precision(reason="f32r compute is intentional"),
        ):
            # constants
            ident = sb.tile([128, 128], F32, tag="ident", bufs=1)
            make_identity(nc, ident[:])
            ident_r = sb.tile([128, 128], F32R, tag="identr", bufs=1)
            nc.vector.tensor_copy(ident_r[:], ident[:])
            ones_f = sb.tile([128, 64], F32, tag="onesf", bufs=1)
            nc.vector.memset(ones_f[:], 1.0)
            ones_b = sb.tile([128, 64], F32R, tag="ones", bufs=1)
            nc.vector.tensor_copy(ones_b[:], ones_f[:])

            # prefetch first x chunks ahead of weight DMAs (queue order)
            _pre_x = {}
            for tci in range(2):
                xs = []
                for tb in range(4):
                    xstg = sb.tile([128, D], F32R, tag="xstg", bufs=8,
                                   name=f"xstg{tci}_{tb}")
                    row0 = (tci * 4 + tb) * 128
                    nc.sync.dma_start(xstg[:], x_d[row0:row0 + 128, :])
                    xs.append(xstg)
                _pre_x[tci] = xs
            # qkv weights: direct DMA into f32r tiles
            w_r = {}
            for w_d, name in ((wq_d, "q"), (wk_d, "k"), (wv_d, "v")):
                tiles = []
                for dc in range(8):
                    wr = sb.tile([128, 128], F32R, tag=f"w{name}", bufs=8)
                    nc.sync.dma_start(wr[:], w_d[dc * 128:(dc + 1) * 128, :])
                    tiles.append(wr)
                w_r[name] = tiles

            # persistent activations
            qT = sb.tile([128, T], F32R, tag="qT", bufs=1)
            kT = sb.tile([128, T], F32R, tag="kT", bufs=1)
            # v in natural layout per head: 32 t-blocks x [ones | 64 v cols]
            v_aug = []
            for h in range(HPC):
                va = sb.tile([128, 32 * 65], F32R, tag=f"vaug{h}", bufs=1)
                ones_cols = va[:].rearrange("p (b c) -> p b c", c=65)[:, :, 64:65]
                nc.vector.tensor_copy(
                    ones_cols,
                    ones_f[:, 0:32].rearrange("p (a b) -> p a b", b=1))
                v_aug.append(va)

            last_obc = [None]

            # ---- phase 1: stream x, transpose, project q/k/v ----
            def load_xstgs(tci):
                xstgs = []
                for tb in range(4):
                    xstg = sb.tile([128, D], F32R, tag="xstg", bufs=8,
                                   name=f"xstg{tci}_{tb}")
                    row0 = (tci * 4 + tb) * 128
                    nc.sync.dma_start(xstg[:], x_d[row0:row0 + 128, :])
                    xstgs.append(xstg)
                return xstgs

            def emit_tchunk(tci, copies_on_act=True, xstgs=None):
                xTc = [sb.tile([128, TCH], F32R, tag="xTc", bufs=10,
                               name=f"xTc{tci}_{d}") for d in range(8)]
                if xstgs is None:
                    xstgs = load_xstgs(tci)
                for dc in range(8):
                    ptr = ps.tile([128, TCH], F32R, tag="ps_a", bufs=2,
                                  name=f"ptr{tci}_{dc}")
                    for tb in range(4):
                        nc.tensor.transpose(
                            ptr[:, tb * 128:(tb + 1) * 128],
                            xstgs[tb][:, dc * 128:(dc + 1) * 128], ident_r[:])
                    if copies_on_act and dc % 2 == 0:
                        nc.scalar.copy(xTc[dc][:], ptr[:])
                    else:
                        nc.vector.tensor_copy(xTc[dc][:], ptr[:])

                for name in ("q", "k", "v"):
                    pp = ps.tile([128, TCH], F32, tag="ps_a", bufs=2,
                                 name=f"pp{tci}_{name}")
                    for dc in range(8):
                        nc.tensor.matmul(pp[:], w_r[name][dc][:], xTc[dc][:],
                                         start=(dc == 0), stop=(dc == 7))
                    col = tci * TCH
                    if name == "q":
                        nc.vector.tensor_copy(qT[:, col:col + TCH], pp[:])
                    elif name == "k":
                        nc.vector.tensor_copy(kT[:, col:col + TCH], pp[:])
                    else:
                        vTs = sb.tile([128, TCH], F32R, tag="vTs", bufs=2,
                                      name=f"vTs{tci}")
                        nc.vector.tensor_copy(vTs[:], pp[:])
                        for h in range(HPC):
                            for tb in range(4):
                                pv = ps.tile([128, 64], F32R, tag="ps_o",
                                             bufs=2, name=f"pv{tci}_{h}_{tb}")
                                with nc.allow_low_precision(
                                        reason="pure transpose"):
                                    nc.tensor.transpose(
                                        pv[:],
                                        vTs[h * 64:(h + 1) * 64,
                                            tb * 128:(tb + 1) * 128],
                                        ident_r[h * 64:(h + 1) * 64,
                                                h * 64:(h + 1) * 64])
                                blk = tci * 4 + tb
                                nc.vector.tensor_copy(
                                    v_aug[h][:, blk * 65:blk * 65 + 64],
                                    pv[:])

            # ---- phase 2: attention unit for (batch, head, q-chunk) ----
            def emit_attn(b, h, qc):
                qoff = b * S + qc * TCH
                po = ps.tile([65, TCH], F32, tag="ps_o", bufs=2,
                             name=f"po{b}_{h}_{qc}")
                for kb2 in range(S // 256):
                    pscr = ps.tile([128, 2 * TCH], F32, tag="ps_s", bufs=2,
                                   name=f"pscr{b}_{h}_{qc}_{kb2}")
                    for j in range(2):
                        kb = 2 * kb2 + j
                        koff = b * S + kb * 128
                        nc.tensor.matmul(
                            pscr[:, j * TCH:(j + 1) * TCH],
                            kT[h * 64:(h + 1) * 64, koff:koff + 128],
                            qT[h * 64:(h + 1) * 64, qoff:qoff + TCH],
                            start=True, stop=True)
                    ex = sb.tile([128, 2 * TCH], F32R, tag="ex", bufs=4,
                                 name=f"ex{b}_{h}_{qc}_{kb2}")
                    nc.scalar.activation(ex[:], pscr[:], EXP, scale=0.125)
                    for j in range(2):
                        kb = 2 * kb2 + j
                        blk = b * 16 + kb
                        nc.tensor.matmul(
                            po[:],
                            v_aug[h][:, blk * 65:blk * 65 + 65],
                            ex[:, j * TCH:(j + 1) * TCH],
                            start=(kb == 0), stop=(kb == S // 128 - 1))
                # normalize: r = 1/sumexp (row 64), broadcast via PE
                r65 = sb.tile([65, TCH], F32R, tag="r", bufs=2,
                              name=f"r{b}_{h}_{qc}")
                nc.vector.reciprocal(r65[64:65, :], po[64:65, :])
                pbc = ps.tile([64, TCH], F32, tag="ps_o", bufs=2,
                              name=f"pbc{b}_{h}_{qc}")
                nc.tensor.matmul(pbc[:], ones_b[64:65, :],
                                 r65[64:65, :], start=True, stop=True)
                bc_sb = sb.tile([64, TCH], F32R, tag="bcsb", bufs=2,
                                name=f"bcsb{b}_{h}_{qc}")
                nc.vector.tensor_copy(bc_sb[:], pbc[:])
                obc = sb.tile([64, TCH], F32R, tag="obc", bufs=3,
                              name=f"obc{b}_{h}_{qc}")
                nc.vector.tensor_mul(obc[:], po[0:64, :], bc_sb[:])
                shard = b * (S // TCH) + qc
                nc.sync.dma_start(bnc_in[h][shard, :, :], obc[:])
                last_obc[0] = obc

            # batch-0 projections first, then interleave batch-0 attention
            # with batch-1 projections
            for tci in range(4):
                emit_tchunk(tci, xstgs=_pre_x.get(tci))
            units_b0 = [(0, h, qc) for h in range(HPC)
                        for qc in range(S // TCH)]
            if phases >= 2:
                for u in units_b0[0:3]:
                    emit_attn(*u)
            for i, tci in enumerate(range(4, 8)):
                emit_tchunk(tci, copies_on_act=False)
                if phases >= 2:
                    for u in units_b0[3 + i * 2:3 + (i + 1) * 2]:
                        emit_attn(*u)
            def emit_a2a(h):
                if collective:
                    nc.gpsimd.collective_compute(
                        "AllToAll", mybir.AluOpType.bypass,
                        replica_groups=[list(range(NCORES))],
                        ins=[bnc_in[h][:]], outs=[bnc_out[h][:]])
                else:
                    nc.sync.dma_start(bnc_out[h][:], bnc_in[h][:])

            if phases >= 2:
                for h in range(HPC):
                    for qc in range(S // TCH):
                        emit_attn(1, h, qc)
                    if phases >= 3 and h == 0:
                        emit_a2a(0)

            # ---- phase 3: A2A head-space -> sequence-space (2nd half) ----
            if phases >= 3:
                emit_a2a(1)
                # keep PE's HAM clock warm across the exposed collective so
                # the output projection starts at 2.4 GHz
                for wi in range(24):
                    wps = ps.tile([64, TCH], F32, tag="ps_a", bufs=2,
                                  name=f"warm{wi}")
                    nc.tensor.matmul(
                        wps[:], last_obc[0][:, 0:64], last_obc[0][:],
                        start=True, stop=True)

            # ---- phase 4: output projection for our 512 rows ----
            phase4 = phases >= 4
            oTf = []
            for hc in range(8 if phase4 else 0):
                t = sb.tile([128, ROWS], F32R, tag="oTf", bufs=8,
                            name=f"oTf{hc}")
                nc.sync.dma_start(t[0:64, :], bnc_out[0][hc, :, :])
                nc.sync.dma_start(t[64:128, :], bnc_out[1][hc, :, :])
                oTf.append(t)
            wo_r = []
            for hc in range(8 if phase4 else 0):
                wr = sb.tile([128, DO], F32R, tag="xstg", bufs=8,
                             name=f"wo{hc}")
                nc.sync.dma_start(wr[:], wo_d[hc * 128:(hc + 1) * 128, :])
                wo_r.append(wr)
            for sb_i in range(ROWS // 128 if phase4 else 0):
                outt = sb.tile([128, DO], F32, tag="osb", bufs=2)
                for doc in range(DO // 512):
                    pout = ps.tile([128, 512], F32, tag="ps_s", bufs=2)
                    for hc in range(8):
                        nc.tensor.matmul(
                            pout[:],
                            oTf[hc][:, sb_i * 128:(sb_i + 1) * 128],
                            wo_r[hc][:, doc * 512:(doc + 1) * 512],
                            start=(hc == 0), stop=(hc == 7))
                    nc.scalar.copy(outt[:, doc * 512:(doc + 1) * 512], pout[:])
                nc.sync.dma_start(out_d[sb_i * 128:(sb_i + 1) * 128, :],
                                  outt[:])

    nc.compile()
    return nc


def _get_nc():
    if "nc" not in _cache:
        _cache["nc"] = _build()
    return _cache["nc"]


def _in_maps(x, Wq, Wk, Wv, Wo):
    x_flat = np.ascontiguousarray(x.reshape(T, D), dtype=np.float32)
    wo = np.ascontiguousarray(Wo, dtype=np.float32)
    maps = []
    for c in range(NCORES):
        h0, h1 = HPC * c, HPC * c + 1
        maps.append({
            "x": x_flat,
            "wq": np.ascontiguousarray(
                np.concatenate([Wq[h0], Wq[h1]], axis=1), dtype=np.float32),
            "wk": np.ascontiguousarray(
                np.concatenate([Wk[h0], Wk[h1]], axis=1), dtype=np.float32),
            "wv": np.ascontiguousarray(
                np.concatenate([Wv[h0], Wv[h1]], axis=1), dtype=np.float32),
            "wo": wo,
        })
    return maps


def kernel(x, Wq, Wk, Wv, Wo, **_):
    nc = _get_nc()
    res = bass_utils.run_bass_kernel_spmd(
        nc, _in_maps(x, Wq, Wk, Wv, Wo), core_ids=list(range(NCORES)))
    out = np.concatenate([res.results[c]["out"] for c in range(NCORES)],
                         axis=0)
    return out.reshape(B, S, DO)
